# revision 14
# baseline (speedup 1.0000x reference)
"""Trainium2 Bass kernel for nn_AST_GAT (gnn_message_passing).

Strategy
--------
The module's output is only ``out[index_map[root_ids]]`` — 64 rows of the
65536-row node state after 20 mean-aggregation SAGE iterations over the
combine-edge forest.  The dependency closure of those 64 rows through the 20
iterations is computed on the host (pure index manipulation: leaf maps, edge
pruning dynamics, reverse BFS).  All sparsity (segment-sums, per-iteration
pruning masks, mean normalization) is folded into small dense selection /
normalized-adjacency matrices, so the device does only dense matmuls.

Sharding: the 64 roots are split 8-per-core (graph-parallel); each core's
closure is independent, so there is no cross-core traffic.  The host gathers
the 8x[8,384] results into the full [64,384] output.

Device program per core (state kept feature-major so no transposes are ever
needed; the self-term and bias are folded into an augmented adjacency matrix
G' = [G; I; 1] applied to the stacked [u; v; bias] block):
  var_x   = vfT.T @ W_lin + b_lin x 1
  s_compT = var_x.T @ McompT         s_combT = subx.T @ McombT
  x0T     = Wl_c.T @ s_compT + Wl_b.T @ s_combT
            + (Wr_c+Wr_b).T @ subxT + (bl_c+bl_b) x 1
  iter i:  u = x @ Wl_lp ; v = x[:PP'] @ Wr_lp   (natural layout, N=384 muls)
           xT' = [u; v; bl_lp].T @ G2T_i          (G2T = [GT; I; 1])
  output: xT20[:, :n_roots] -> host transposes.
"""
import sys

sys.path.insert(0, "/opt/trn_rl_repo")

import numpy as np

N_ITERS = 20
N_CORES = 8
D = 384
KC = 3  # 128-chunks of D
P = 128

F32 = np.float32
F16 = np.float16  # matmul operand dtype: fp16 streams 1 row/cycle on the PE
                  # (vs 4 for fp32) and halves DMA bytes; PSUM accumulates fp32


# ======================================================================
# Host-side preprocessing (faithful numpy reimplementation of the
# reference's index semantics + dependency closure of the root rows).
# ======================================================================

def _append_unique(order, pos, items):
    for s in items:
        s = int(s)
        if s not in pos:
            pos[s] = len(order)
            order.append(s)


def graph_prep(E, root_ids, n_var, n_sub):
    src = np.asarray(E[0], dtype=np.int64)
    dst = np.asarray(E[1], dtype=np.int64)
    root_ids = np.asarray(root_ids, dtype=np.int64)
    N = n_var + n_sub

    leaf_mask = np.ones(N, dtype=bool)
    leaf_mask[dst] = False
    leaf_idx = np.cumsum(leaf_mask) - 1
    nleaf_idx = np.cumsum(~leaf_mask) - 1
    index_map = np.where(leaf_mask, leaf_idx, nleaf_idx)

    src_is_leaf = leaf_mask[src]
    src_loc = index_map[src]
    dst_loc = index_map[dst]
    src_s = np.clip(src_loc, 0, n_sub - 1)

    # simulate the pruning dynamics exactly as the reference scan does
    actives, dones = [], []
    active = ~src_is_leaf
    done = False
    for _ in range(N_ITERS):
        actives.append(active.copy())
        dones.append(done)
        idx = np.where(active, dst_loc, n_sub)
        valid = (idx >= 0) & (idx <= n_sub)  # jax scatter drops OOB
        is_tgt = np.zeros(n_sub + 1, dtype=bool)
        is_tgt[idx[valid]] = True
        active_new = active & is_tgt[src_s]
        if not done:
            active = active_new
        done = done or (not active.any())

    dmask = (dst_loc >= 0) & (dst_loc < n_sub)
    cnts = []
    for i in range(N_ITERS):
        if dones[i]:
            cnts.append(None)
        else:
            d = dst_loc[actives[i] & dmask]
            cnts.append(np.bincount(d, minlength=n_sub).astype(np.float64))

    return dict(
        src_is_leaf=src_is_leaf, src_loc=src_loc, dst_loc=dst_loc, src_s=src_s,
        actives=actives, dones=dones, cnts=cnts, dmask=dmask,
        j_roots=index_map[root_ids], n_sub=n_sub,
    )


def core_closure(g, roots):
    dst_loc, src_s = g["dst_loc"], g["src_s"]
    actives, dones, n_sub, dmask = g["actives"], g["dones"], g["n_sub"], g["dmask"]

    order, pos = [], {}
    _append_unique(order, pos, roots)
    lens = [0] * (N_ITERS + 1)
    lens[N_ITERS] = len(order)

    member = np.zeros(n_sub, dtype=bool)
    member[order] = True

    for i in range(N_ITERS - 1, -1, -1):
        if not dones[i]:
            e = actives[i] & dmask
            e[e] = member[dst_loc[e]]
            fresh = np.unique(src_s[e])
            fresh = fresh[~member[fresh]]
            _append_unique(order, pos, np.sort(fresh))
            member[fresh] = True
        lens[i] = len(order)

    posarr = np.full(n_sub, -1, dtype=np.int64)
    order_arr = np.array(order, dtype=np.int64)
    posarr[order_arr] = np.arange(len(order))

    in0 = dmask.copy()
    in0[dmask] = member[dst_loc[dmask]]

    comp_e = in0 & g["src_is_leaf"]
    leaves = g["src_loc"][comp_e]
    leaf_order, leaf_pos = [], {}
    _append_unique(leaf_order, leaf_pos, leaves)

    comb_e = in0 & ~g["src_is_leaf"]
    subs = np.clip(g["src_loc"][comb_e], 0, n_sub - 1)  # jax gather clamps
    sub_order, sub_pos = list(order), dict(pos)
    _append_unique(sub_order, sub_pos, subs)

    return dict(
        order=order_arr, pos=pos, posarr=posarr, lens=lens,
        comp_e=comp_e, leaves=leaves, leaf_order=np.array(leaf_order, np.int64),
        leaf_pos=leaf_pos,
        comb_e=comb_e, subs=subs, sub_order=np.array(sub_order, np.int64),
        sub_pos=sub_pos,
    )


def build_core_problem(g, cl, inputs, PPs, Lp, Pb):
    dst_loc, src_s = g["dst_loc"], g["src_s"]
    actives, dones, cnts, dmask = g["actives"], g["dones"], g["cnts"], g["dmask"]
    posarr, lens = cl["posarr"], cl["lens"]
    n0, PP0 = lens[0], PPs[0]
    n_sub = g["n_sub"]

    McompT = np.zeros((Lp, PP0), dtype=F32)
    if cl["leaves"].size:
        lcols = np.array([cl["leaf_pos"][int(s)] for s in cl["leaves"]], np.int64)
        np.add.at(McompT, (lcols, posarr[dst_loc[cl["comp_e"]]]), 1.0)

    McombT = np.zeros((Pb, PP0), dtype=F32)
    if cl["subs"].size:
        scols = np.array([cl["sub_pos"][int(s)] for s in cl["subs"]], np.int64)
        np.add.at(McombT, (scols, posarr[dst_loc[cl["comb_e"]]]), 1.0)

    GTs = []
    for i in range(N_ITERS):
        if dones[i]:
            GTs.append(None)
            continue
        nip1 = lens[i + 1]
        e = actives[i] & dmask
        sel = e.copy()
        p = posarr[dst_loc[e]]
        sel[e] = (p >= 0) & (p < nip1)
        GT = np.zeros((PPs[i], PPs[i + 1]), dtype=F32)
        if sel.any():
            rows = posarr[dst_loc[sel]]
            cols = posarr[src_s[sel]]
            vals = (1.0 / np.maximum(cnts[i][dst_loc[sel]], 1.0)).astype(F32)
            np.add.at(GT, (cols, rows), vals)
        GTs.append(GT)

    var_feats = np.asarray(inputs["var_feats"], dtype=F32)
    code_emb = np.asarray(inputs["code_emb"], dtype=F32)
    sids = np.clip(np.asarray(inputs["subcode_ids"], dtype=np.int64), 0,
                   code_emb.shape[0] - 1)

    vfT = np.zeros((D, Lp), dtype=F32)
    lo = cl["leaf_order"]
    if lo.size:
        vfT[:, : lo.size] = var_feats[lo].T
    subx = np.zeros((Pb, D), dtype=F32)
    so = cl["sub_order"]
    subx[: so.size] = code_emb[sids[so]]
    subxT_pref = np.zeros((D, PP0), dtype=F32)
    subxT_pref[:, :n0] = subx[:n0].T

    return dict(vfT=vfT, subx=subx, subxT=subxT_pref,
                mcompT=McompT, mcombT=McombT, GTs=GTs)


def preprocess(inputs):
    n_var = inputs["var_feats"].shape[0]
    n_sub = inputs["subcode_ids"].shape[0]
    root_ids = np.asarray(inputs["root_ids"], dtype=np.int64)
    B = root_ids.shape[0]
    assert B % N_CORES == 0
    per_core = B // N_CORES

    g = graph_prep(np.asarray(inputs["E"]), root_ids, n_var, n_sub)
    closures = [core_closure(g, g["j_roots"][c * per_core:(c + 1) * per_core])
                for c in range(N_CORES)]

    # pad to 32 so partition starts of the stacked [u; v; bias] segments are
    # 32-aligned (SBUF AP constraint) and no uninitialized gap rows exist
    PPs = [-(-max(cl["lens"][i] for cl in closures) // 32) * 32
           for i in range(N_ITERS + 1)]
    Lp = max(max(cl["leaf_order"].size for cl in closures), 1)
    Pb = max(max(cl["sub_order"].size for cl in closures), 1)

    probs = [build_core_problem(g, cl, inputs, PPs, Lp, Pb) for cl in closures]

    out_map = []
    for r in range(B):
        c = r // per_core
        j = int(g["j_roots"][r])
        out_map.append((c, closures[c]["pos"][j]))

    live = [i for i in range(N_ITERS) if not g["dones"][i]]
    return dict(probs=probs, PPs=PPs, Lp=Lp, Pb=Pb, out_map=out_map, live=live)


def stack_g2(GT, PPi, PPn):
    """Augmented adjacency: rows [0,PPi) = GT, [PPi,PPi+PPn) = identity
    (self term), plus a trailing ones row (bias) unless PPi+PPn is a
    multiple of 128 (then the bias is applied as a rank-1 matmul on the
    v psum instead, to avoid a 1-row extra partition chunk)."""
    bias_row = (PPi + PPn) % P != 0
    SP = PPi + PPn + (1 if bias_row else 0)
    G2 = np.zeros((SP, PPn), dtype=F32)
    G2[:PPi] = GT
    G2[PPi:PPi + PPn, :][np.arange(PPn), np.arange(PPn)] = 1.0
    if bias_row:
        G2[SP - 1, :] = 1.0
    return G2


def pack_rows(a):
    """[R, C] -> [P, ceil(R/P)*C]: 128-row chunks laid side by side, so a
    whole tensor lands in SBUF with ONE dma_start (DMA issue on the engines
    is serial and ~600ns per descriptor — fewer, bigger transfers win)."""
    R, C = a.shape
    nch = -(-R // P)
    out = np.zeros((P, nch * C), a.dtype)
    for c in range(nch):
        r0, r1 = c * P, min((c + 1) * P, R)
        out[: r1 - r0, c * C:(c + 1) * C] = a[r0:r1]
    return out


# ======================================================================
# Device program
# ======================================================================

def _chunks(n):
    return [(s, min(P, n - s)) for s in range(0, n, P)]


def build_program(PPs, Lp, Pb, live):
    import concourse.mybir as mybir
    import concourse.tile as tile
    from concourse import bacc

    f32 = mybir.dt.float32
    mdt = mybir.dt.float16
    PP0 = PPs[0]
    OUTW = max(PPs[N_ITERS], 1)

    nc = bacc.Bacc("TRN2", target_bir_lowering=False, debug=False,
                   num_devices=N_CORES)

    Lch = -(-Lp // P)
    Pbch = -(-Pb // P)

    # ---- DRAM parameters (all pre-chunked to [P, nch*C] by pack_rows) ----
    wnames = ["wlin", "wlc", "wlb", "wrsum", "wllp", "wrlp"]
    wd = {n: nc.declare_dram_parameter(n, [P, KC * D], mdt, isOutput=False)
          for n in wnames}
    bnames = ["blin", "blcb", "bllp"]
    bd = {n: nc.declare_dram_parameter(n, [1, D], mdt, isOutput=False)
          for n in bnames}
    ones_d = nc.declare_dram_parameter("ones", [1, 512], mdt, isOutput=False)
    vfT_d = nc.declare_dram_parameter("vfT", [P, KC * Lp], mdt, isOutput=False)
    subx_d = nc.declare_dram_parameter("subx", [P, Pbch * D], mdt, isOutput=False)
    subxT_d = nc.declare_dram_parameter("subxT", [P, KC * PP0], mdt,
                                        isOutput=False)
    mcompT_d = nc.declare_dram_parameter("mcompT", [P, Lch * PP0], mdt,
                                         isOutput=False)
    mcombT_d = nc.declare_dram_parameter("mcombT", [P, Pbch * PP0], mdt,
                                         isOutput=False)
    gt_d = {}
    for i in live:
        SP = PPs[i] + PPs[i + 1]
        if SP % P != 0:
            SP += 1
        nch = -(-SP // P)
        gt_d[i] = nc.declare_dram_parameter(f"gt{i}", [P, nch * PPs[i + 1]],
                                            mdt, isOutput=False)
    out_d = nc.declare_dram_parameter("out", [KC, P, OUTW], f32, isOutput=True)

    with tile.TileContext(nc) as tc:
        with (
            tc.tile_pool(name="const", bufs=1) as const,
            tc.tile_pool(name="state", bufs=2) as state,
            tc.tile_pool(name="ps", bufs=4, space="PSUM") as ps,
        ):
            # ---- load constants, in first-use order ----
            # one dma_start per (packed) tensor, spread over the three
            # DMA-capable queues: sync + scalar (HWDGE) and gpsimd (SWDGE).
            # critical path (varx needs wlin/blin/ones/vfT) goes on sync.
            wt = {n: const.tile([P, KC * D], mdt, tag=n, name=n)
                  for n in wnames}
            bt = {n: const.tile([1, D], mdt, tag=n, name=n) for n in bnames}
            ones = const.tile([1, 512], mdt, tag="ones")

            nc.sync.dma_start(out=ones[:], in_=ones_d[:])
            # HAM warm-up: ~3.5us of dummy PE work issued while the
            # constant DMAs stream, so real matmuls start at 2.4 GHz
            for _ in range(8):
                wpt = ps.tile([P, 64], f32, tag="small_ps", name="warm")
                nc.tensor.matmul(out=wpt[:, :], lhsT=ones[:1, :P],
                                 rhs=ones[:1, :64], start=True, stop=True)
            nc.sync.dma_start(out=wt["wlin"][:], in_=wd["wlin"][:])
            nc.sync.dma_start(out=bt["blin"][:], in_=bd["blin"][:])
            vf_t = const.tile([P, KC * Lp], mdt, tag="vfT")
            nc.sync.dma_start(out=vf_t[:], in_=vfT_d[:])

            mcompT_t = const.tile([P, Lch * PP0], mdt, tag="mcompT")
            nc.scalar.dma_start(out=mcompT_t[:], in_=mcompT_d[:])
            subx_t = const.tile([P, Pbch * D], mdt, tag="subx")
            nc.scalar.dma_start(out=subx_t[:], in_=subx_d[:])
            mcombT_t = const.tile([P, Pbch * PP0], mdt, tag="mcombT")
            nc.scalar.dma_start(out=mcombT_t[:], in_=mcombT_d[:])
            for n in ("wlc", "wlb", "wrsum"):
                nc.scalar.dma_start(out=wt[n][:], in_=wd[n][:])
            nc.scalar.dma_start(out=bt["blcb"][:], in_=bd["blcb"][:])
            subxT_t = const.tile([P, KC * PP0], mdt, tag="subxT")
            nc.scalar.dma_start(out=subxT_t[:], in_=subxT_d[:])

            nc.gpsimd.dma_start(out=wt["wllp"][:], in_=wd["wllp"][:])
            nc.gpsimd.dma_start(out=wt["wrlp"][:], in_=wd["wrlp"][:])
            nc.gpsimd.dma_start(out=bt["bllp"][:], in_=bd["bllp"][:])
            gt_t = {}
            for i in live:
                SP = PPs[i] + PPs[i + 1]
                if SP % P != 0:
                    SP += 1
                nch = -(-SP // P)
                t = const.tile([P, nch * PPs[i + 1]], mdt, tag=f"gt{i}")
                nc.gpsimd.dma_start(out=t[:], in_=gt_d[i][:])
                gt_t[i] = t

            # PSUM-read copies round-robin between the vector (DVE) and
            # scalar (Activation) engines so chunk copies run concurrently.
            _cp = [0]

            def pcopy(out, in_):
                _cp[0] += 1
                if _cp[0] % 2:
                    nc.vector.tensor_copy(out=out, in_=in_)
                else:
                    nc.scalar.copy(out, in_)

            # ---- var_x = vfT.T @ W_lin + b_lin ----
            varx_t = []
            for ci, (s, sz) in enumerate(_chunks(Lp)):
                pt = ps.tile([P, D], f32, tag="big_ps")
                for k in range(KC):
                    nc.tensor.matmul(out=pt[:sz, :],
                                     lhsT=vf_t[:, k * Lp + s:k * Lp + s + sz],
                                     rhs=wt["wlin"][:, k * D:(k + 1) * D],
                                     start=(k == 0), stop=False)
                nc.tensor.matmul(out=pt[:sz, :], lhsT=ones[:1, :sz],
                                 rhs=bt["blin"][:1, :], start=False, stop=True)
                t = state.tile([P, D], mdt, tag=f"varx{ci}")
                pcopy(t[:sz, :], pt[:sz, :])
                varx_t.append(t)

            # ---- s_compT / s_combT (feature-major) ----
            scompT_t, scombT_t = [], []
            lpch = _chunks(Lp)
            pbch = _chunks(Pb)
            for dk in range(KC):
                pt = ps.tile([P, PP0], f32, tag="small_ps")
                for ci, (s, sz) in enumerate(lpch):
                    nc.tensor.matmul(out=pt[:, :],
                                     lhsT=varx_t[ci][:sz, dk * P:(dk + 1) * P],
                                     rhs=mcompT_t[:sz, ci * PP0:(ci + 1) * PP0],
                                     start=(ci == 0), stop=(ci == len(lpch) - 1))
                t = state.tile([P, PP0], mdt, tag=f"scompT{dk}")
                pcopy(t[:], pt[:])
                scompT_t.append(t)
            for dk in range(KC):
                pt = ps.tile([P, PP0], f32, tag="small_ps")
                for ci, (s, sz) in enumerate(pbch):
                    nc.tensor.matmul(out=pt[:, :],
                                     lhsT=subx_t[:sz, ci * D + dk * P:ci * D + (dk + 1) * P],
                                     rhs=mcombT_t[:sz, ci * PP0:(ci + 1) * PP0],
                                     start=(ci == 0), stop=(ci == len(pbch) - 1))
                t = state.tile([P, PP0], mdt, tag=f"scombT{dk}")
                pcopy(t[:], pt[:])
                scombT_t.append(t)

            # ---- x0T (one [P, KC*PP0] tile; dk chunks side by side) ----
            xTt = state.tile([P, KC * PP0], mdt, tag="xT_a")
            xw = PP0
            for dk in range(KC):
                pt = ps.tile([P, PP0], f32, tag="small_ps")
                for k in range(KC):
                    nc.tensor.matmul(out=pt[:, :],
                                     lhsT=wt["wlc"][:, k * D + dk * P:k * D + (dk + 1) * P],
                                     rhs=scompT_t[k][:, :],
                                     start=(k == 0), stop=False)
                for k in range(KC):
                    nc.tensor.matmul(out=pt[:, :],
                                     lhsT=wt["wlb"][:, k * D + dk * P:k * D + (dk + 1) * P],
                                     rhs=scombT_t[k][:, :],
                                     start=False, stop=False)
                for k in range(KC):
                    nc.tensor.matmul(out=pt[:, :],
                                     lhsT=wt["wrsum"][:, k * D + dk * P:k * D + (dk + 1) * P],
                                     rhs=subxT_t[:, k * PP0:(k + 1) * PP0],
                                     start=False, stop=False)
                nc.tensor.matmul(out=pt[:, :],
                                 lhsT=bt["blcb"][:1, dk * P:(dk + 1) * P],
                                 rhs=ones[:1, :PP0], start=False, stop=True)
                pcopy(xTt[:, dk * PP0:(dk + 1) * PP0], pt[:])

            # ---- iterations ----
            for step, i in enumerate(live):
                PPi, PPn = PPs[i], PPs[i + 1]
                bias_row = (PPi + PPn) % P != 0
                SP = PPi + PPn + (1 if bias_row else 0)
                sch = _chunks(SP)
                us = [state.tile([P, D], mdt, tag=f"us{ci}", name=f"us{ci}_{step}")
                      for ci in range(len(sch))]

                def allowed(off):
                    # SBUF partition-start quadrant rule
                    return {0: P, 32: 32, 64: 64, 96: 32}[off]

                def copy_rows(gstart, pt, nrows):
                    r = 0
                    while r < nrows:
                        g0 = gstart + r
                        ci, off = g0 // P, g0 % P
                        take = min(allowed(off), allowed(r % P), nrows - r)
                        pcopy(us[ci][off:off + take, :], pt[r:r + take, :])
                        r += take

                if SP <= P:
                    # single merged psum: u rows [0,PPi), v rows
                    # [PPi,PPi+PPn), bias row last -> one copy
                    pt = ps.tile([P, D], f32, tag="big_ps")
                    for k in range(KC):
                        nc.tensor.matmul(out=pt[:PPi, :],
                                         lhsT=xTt[:, k * xw:k * xw + PPi],
                                         rhs=wt["wllp"][:, k * D:(k + 1) * D],
                                         start=(k == 0), stop=(k == KC - 1))
                    for k in range(KC):
                        nc.tensor.matmul(out=pt[PPi:PPi + PPn, :],
                                         lhsT=xTt[:, k * xw:k * xw + PPn],
                                         rhs=wt["wrlp"][:, k * D:(k + 1) * D],
                                         start=(k == 0),
                                         stop=(k == KC - 1 and bias_row))
                    bias_dve = False
                    if bias_row:
                        if (SP - 1) % P in (0, 32, 64):
                            nc.tensor.matmul(out=pt[SP - 1:SP, :],
                                             lhsT=ones[:1, :1],
                                             rhs=bt["bllp"][:1, :],
                                             start=True, stop=True)
                        else:  # matmul out base partition must be 0/32/64
                            bias_dve = True
                    else:
                        nc.tensor.matmul(out=pt[PPi:PPi + PPn, :],
                                         lhsT=ones[:1, :PPn],
                                         rhs=bt["bllp"][:1, :],
                                         start=False, stop=True)
                    ncp = SP - 1 if bias_dve else SP
                    pcopy(us[0][:ncp, :], pt[:ncp, :])
                    if bias_dve:
                        nc.gpsimd.tensor_copy(out=us[0][SP - 1:SP, :],
                                              in_=bt["bllp"][:1, :])
                else:
                    for (s, sz) in _chunks(PPi):
                        pt = ps.tile([P, D], f32, tag="big_ps")
                        for k in range(KC):
                            nc.tensor.matmul(out=pt[:sz, :],
                                             lhsT=xTt[:, k * xw + s:k * xw + s + sz],
                                             rhs=wt["wllp"][:, k * D:(k + 1) * D],
                                             start=(k == 0), stop=(k == KC - 1))
                        copy_rows(s, pt, sz)
                    for (s, sz) in _chunks(PPn):
                        pt = ps.tile([P, D], f32, tag="big_ps")
                        for k in range(KC):
                            nc.tensor.matmul(out=pt[:sz, :],
                                             lhsT=xTt[:, k * xw + s:k * xw + s + sz],
                                             rhs=wt["wrlp"][:, k * D:(k + 1) * D],
                                             start=(k == 0),
                                             stop=(k == KC - 1 and bias_row))
                        # without a stacked bias row, fold the bias into
                        # the v psum as a rank-1 matmul
                        if not bias_row:
                            nc.tensor.matmul(out=pt[:sz, :],
                                             lhsT=ones[:1, :sz],
                                             rhs=bt["bllp"][:1, :],
                                             start=False, stop=True)
                        copy_rows(PPi + s, pt, sz)
                    if bias_row:
                        gb = SP - 1
                        nc.gpsimd.tensor_copy(
                            out=us[gb // P][gb % P:gb % P + 1, :],
                            in_=bt["bllp"][:1, :])

                ab = "ab"[step % 2]
                # final step's tiles feed the f32 output DMA directly
                odt = f32 if step == len(live) - 1 else mdt
                # one psum bank holds all KC dk-chunks -> one (split) copy
                W = KC * PPn
                pt = ps.tile([P, W], f32, tag="small_ps")
                for dk in range(KC):
                    for ci, (s, sz) in enumerate(sch):
                        nc.tensor.matmul(out=pt[:, dk * PPn:(dk + 1) * PPn],
                                         lhsT=us[ci][:sz, dk * P:(dk + 1) * P],
                                         rhs=gt_t[i][:sz, ci * PPn:(ci + 1) * PPn],
                                         start=(ci == 0), stop=(ci == len(sch) - 1))
                xTt = state.tile([P, W], odt, tag=f"xT_{ab}")
                xw = PPn
                half = 2 * PPn  # split on a dk boundary: 2/3 vector, 1/3 scalar
                nc.vector.tensor_copy(out=xTt[:, :half], in_=pt[:, :half])
                nc.scalar.copy(xTt[:, half:], pt[:, half:])

            # ---- output ----
            for dk in range(KC):
                nc.sync.dma_start(out=out_d[dk],
                                  in_=xTt[:, dk * xw:dk * xw + OUTW])

    nc.compile()
    return nc


# ======================================================================
# Entry point
# ======================================================================

def kernel(**inputs) -> np.ndarray:
    out, _ = _run(inputs)
    return out


def _run(inputs, **spmd_kwargs):
    from concourse.bass_utils import run_bass_kernel_spmd

    pre = preprocess(inputs)
    PPs, Lp, Pb, live = pre["PPs"], pre["Lp"], pre["Pb"], pre["live"]

    nc = build_program(PPs, Lp, Pb, live)

    def f(a):
        return np.ascontiguousarray(np.asarray(a, F32))

    def h(a):
        return np.ascontiguousarray(np.asarray(a, F16))

    shared = {
        "wlin": h(inputs["W_lin"]), "wlc": h(inputs["Wl_c"]),
        "wlb": h(inputs["Wl_b"]),
        "wrsum": h(f(inputs["Wr_c"]) + f(inputs["Wr_b"])),
        "wllp": h(inputs["Wl_lp"]), "wrlp": h(inputs["Wr_lp"]),
        "blin": h(inputs["b_lin"]).reshape(1, D),
        "blcb": h(f(inputs["bl_c"]) + f(inputs["bl_b"])).reshape(1, D),
        "bllp": h(inputs["bl_lp"]).reshape(1, D),
        "ones": np.ones((1, 512), dtype=F16),
    }

    for k in ("wlin", "wlc", "wlb", "wrsum", "wllp", "wrlp"):
        shared[k] = pack_rows(shared[k])

    in_maps = []
    for c in range(N_CORES):
        prob = pre["probs"][c]
        m = dict(shared)
        m["vfT"] = pack_rows(h(prob["vfT"]))
        m["subx"] = pack_rows(h(prob["subx"]))
        m["subxT"] = pack_rows(h(prob["subxT"]))
        m["mcompT"] = pack_rows(h(prob["mcompT"]))
        m["mcombT"] = pack_rows(h(prob["mcombT"]))
        for i in live:
            g2 = stack_g2(prob["GTs"][i], PPs[i], PPs[i + 1])
            m[f"gt{i}"] = pack_rows(h(g2))
        in_maps.append(m)

    res = run_bass_kernel_spmd(nc, in_maps, core_ids=list(range(N_CORES)),
                               **spmd_kwargs)

    B = len(pre["out_map"])
    OUTW = max(PPs[N_ITERS], 1)
    out = np.zeros((B, D), dtype=F32)
    for r, (c, row) in enumerate(pre["out_map"]):
        o = res.results[c]["out"].reshape(KC * P, OUTW)  # [384, OUTW]
        out[r] = o[:, row]
    return out, res



# revision 21
# speedup vs baseline: 1.1074x; 1.1074x over previous
"""Trainium2 Bass kernel for nn_AST_GAT (gnn_message_passing).

Strategy
--------
The module's output is only ``out[index_map[root_ids]]`` — 64 rows of the
65536-row node state after 20 mean-aggregation SAGE iterations over the
combine-edge forest.  The dependency closure of those 64 rows through the 20
iterations is computed on the host (pure index manipulation: leaf maps, edge
pruning dynamics, reverse BFS).  All sparsity (segment-sums, per-iteration
pruning masks, mean normalization) is folded into small dense selection /
normalized-adjacency matrices, so the device does only dense matmuls.

Sharding: the 64 roots are split 8-per-core (graph-parallel); each core's
closure is independent, so there is no cross-core traffic.  The host gathers
the 8x[8,384] results into the full [64,384] output.

Device program per core (state kept feature-major so no transposes are ever
needed; the self-term and bias are folded into an augmented adjacency matrix
G' = [G; I; 1] applied to the stacked [u; v; bias] block):
  var_x   = vfT.T @ W_lin + b_lin x 1
  s_compT = var_x.T @ McompT         s_combT = subx.T @ McombT
  x0T     = Wl_c.T @ s_compT + Wl_b.T @ s_combT
            + (Wr_c+Wr_b).T @ subxT + (bl_c+bl_b) x 1
  iter i:  u = x @ Wl_lp ; v = x[:PP'] @ Wr_lp   (natural layout, N=384 muls)
           xT' = [u; v; bl_lp].T @ G2T_i          (G2T = [GT; I; 1])
  output: xT20[:, :n_roots] -> host transposes.
"""
import sys

sys.path.insert(0, "/opt/trn_rl_repo")

import numpy as np

N_ITERS = 20
N_CORES = 8
D = 384
KC = 3  # 128-chunks of D
P = 128

F32 = np.float32
F16 = np.float16  # matmul operand dtype: fp16 streams 1 row/cycle on the PE
                  # (vs 4 for fp32) and halves DMA bytes; PSUM accumulates fp32


# ======================================================================
# Host-side preprocessing (faithful numpy reimplementation of the
# reference's index semantics + dependency closure of the root rows).
# ======================================================================

def _append_unique(order, pos, items):
    for s in items:
        s = int(s)
        if s not in pos:
            pos[s] = len(order)
            order.append(s)


def graph_prep(E, root_ids, n_var, n_sub):
    src = np.asarray(E[0], dtype=np.int64)
    dst = np.asarray(E[1], dtype=np.int64)
    root_ids = np.asarray(root_ids, dtype=np.int64)
    N = n_var + n_sub

    leaf_mask = np.ones(N, dtype=bool)
    leaf_mask[dst] = False
    leaf_idx = np.cumsum(leaf_mask) - 1
    nleaf_idx = np.cumsum(~leaf_mask) - 1
    index_map = np.where(leaf_mask, leaf_idx, nleaf_idx)

    src_is_leaf = leaf_mask[src]
    src_loc = index_map[src]
    dst_loc = index_map[dst]
    src_s = np.clip(src_loc, 0, n_sub - 1)

    # simulate the pruning dynamics exactly as the reference scan does
    actives, dones = [], []
    active = ~src_is_leaf
    done = False
    for _ in range(N_ITERS):
        actives.append(active.copy())
        dones.append(done)
        idx = np.where(active, dst_loc, n_sub)
        valid = (idx >= 0) & (idx <= n_sub)  # jax scatter drops OOB
        is_tgt = np.zeros(n_sub + 1, dtype=bool)
        is_tgt[idx[valid]] = True
        active_new = active & is_tgt[src_s]
        if not done:
            active = active_new
        done = done or (not active.any())

    dmask = (dst_loc >= 0) & (dst_loc < n_sub)
    cnts = []
    for i in range(N_ITERS):
        if dones[i]:
            cnts.append(None)
        else:
            d = dst_loc[actives[i] & dmask]
            cnts.append(np.bincount(d, minlength=n_sub).astype(np.float64))

    return dict(
        src_is_leaf=src_is_leaf, src_loc=src_loc, dst_loc=dst_loc, src_s=src_s,
        actives=actives, dones=dones, cnts=cnts, dmask=dmask,
        j_roots=index_map[root_ids], n_sub=n_sub,
    )


def core_closure(g, roots):
    dst_loc, src_s = g["dst_loc"], g["src_s"]
    actives, dones, n_sub, dmask = g["actives"], g["dones"], g["n_sub"], g["dmask"]

    order, pos = [], {}
    _append_unique(order, pos, roots)
    lens = [0] * (N_ITERS + 1)
    lens[N_ITERS] = len(order)

    member = np.zeros(n_sub, dtype=bool)
    member[order] = True

    for i in range(N_ITERS - 1, -1, -1):
        if not dones[i]:
            e = actives[i] & dmask
            e[e] = member[dst_loc[e]]
            fresh = np.unique(src_s[e])
            fresh = fresh[~member[fresh]]
            _append_unique(order, pos, np.sort(fresh))
            member[fresh] = True
        lens[i] = len(order)

    posarr = np.full(n_sub, -1, dtype=np.int64)
    order_arr = np.array(order, dtype=np.int64)
    posarr[order_arr] = np.arange(len(order))

    in0 = dmask.copy()
    in0[dmask] = member[dst_loc[dmask]]

    comp_e = in0 & g["src_is_leaf"]
    leaves = g["src_loc"][comp_e]
    leaf_order, leaf_pos = [], {}
    _append_unique(leaf_order, leaf_pos, leaves)

    comb_e = in0 & ~g["src_is_leaf"]
    subs = np.clip(g["src_loc"][comb_e], 0, n_sub - 1)  # jax gather clamps
    sub_order, sub_pos = list(order), dict(pos)
    _append_unique(sub_order, sub_pos, subs)

    return dict(
        order=order_arr, pos=pos, posarr=posarr, lens=lens,
        comp_e=comp_e, leaves=leaves, leaf_order=np.array(leaf_order, np.int64),
        leaf_pos=leaf_pos,
        comb_e=comb_e, subs=subs, sub_order=np.array(sub_order, np.int64),
        sub_pos=sub_pos,
    )


def build_core_problem(g, cl, inputs, PPs, Lp, Pb):
    dst_loc, src_s = g["dst_loc"], g["src_s"]
    actives, dones, cnts, dmask = g["actives"], g["dones"], g["cnts"], g["dmask"]
    posarr, lens = cl["posarr"], cl["lens"]
    n0, PP0 = lens[0], PPs[0]
    n_sub = g["n_sub"]

    McompT = np.zeros((Lp, PP0), dtype=F32)
    if cl["leaves"].size:
        lcols = np.array([cl["leaf_pos"][int(s)] for s in cl["leaves"]], np.int64)
        np.add.at(McompT, (lcols, posarr[dst_loc[cl["comp_e"]]]), 1.0)

    McombT = np.zeros((Pb, PP0), dtype=F32)
    if cl["subs"].size:
        scols = np.array([cl["sub_pos"][int(s)] for s in cl["subs"]], np.int64)
        np.add.at(McombT, (scols, posarr[dst_loc[cl["comb_e"]]]), 1.0)

    GTs = []
    for i in range(N_ITERS):
        if dones[i]:
            GTs.append(None)
            continue
        nip1 = lens[i + 1]
        e = actives[i] & dmask
        sel = e.copy()
        p = posarr[dst_loc[e]]
        sel[e] = (p >= 0) & (p < nip1)
        GT = np.zeros((PPs[i], PPs[i + 1]), dtype=F32)
        if sel.any():
            rows = posarr[dst_loc[sel]]
            cols = posarr[src_s[sel]]
            vals = (1.0 / np.maximum(cnts[i][dst_loc[sel]], 1.0)).astype(F32)
            np.add.at(GT, (cols, rows), vals)
        GTs.append(GT)

    var_feats = np.asarray(inputs["var_feats"], dtype=F32)
    code_emb = np.asarray(inputs["code_emb"], dtype=F32)
    sids = np.clip(np.asarray(inputs["subcode_ids"], dtype=np.int64), 0,
                   code_emb.shape[0] - 1)

    vfT = np.zeros((D, Lp), dtype=F32)
    lo = cl["leaf_order"]
    if lo.size:
        vfT[:, : lo.size] = var_feats[lo].T
    subx = np.zeros((Pb, D), dtype=F32)
    so = cl["sub_order"]
    subx[: so.size] = code_emb[sids[so]]
    subxT_pref = np.zeros((D, PP0), dtype=F32)
    subxT_pref[:, :n0] = subx[:n0].T

    return dict(vfT=vfT, subx=subx, subxT=subxT_pref,
                mcompT=McompT, mcombT=McombT, GTs=GTs)


def preprocess(inputs):
    n_var = inputs["var_feats"].shape[0]
    n_sub = inputs["subcode_ids"].shape[0]
    root_ids = np.asarray(inputs["root_ids"], dtype=np.int64)
    B = root_ids.shape[0]
    assert B % N_CORES == 0
    per_core = B // N_CORES

    g = graph_prep(np.asarray(inputs["E"]), root_ids, n_var, n_sub)
    closures = [core_closure(g, g["j_roots"][c * per_core:(c + 1) * per_core])
                for c in range(N_CORES)]

    # no padding needed: every copy/matmul now starts at partition 0
    PPs = [max(cl["lens"][i] for cl in closures) for i in range(N_ITERS + 1)]
    Lp = max(max(cl["leaf_order"].size for cl in closures), 1)
    Pb = max(max(cl["sub_order"].size for cl in closures), 1)

    probs = [build_core_problem(g, cl, inputs, PPs, Lp, Pb) for cl in closures]

    out_map = []
    for r in range(B):
        c = r // per_core
        j = int(g["j_roots"][r])
        out_map.append((c, closures[c]["pos"][j]))

    live = [i for i in range(N_ITERS) if not g["dones"][i]]
    return dict(probs=probs, PPs=PPs, Lp=Lp, Pb=Pb, out_map=out_map, live=live)


def pack_rows(a):
    """[R, C] -> [P, ceil(R/P)*C]: 128-row chunks laid side by side, so a
    whole tensor lands in SBUF with ONE dma_start (DMA issue on the engines
    is serial and ~600ns per descriptor — fewer, bigger transfers win)."""
    R, C = a.shape
    nch = -(-R // P)
    out = np.zeros((P, nch * C), a.dtype)
    for c in range(nch):
        r0, r1 = c * P, min((c + 1) * P, R)
        out[: r1 - r0, c * C:(c + 1) * C] = a[r0:r1]
    return out


# ======================================================================
# Device program
# ======================================================================

def _chunks(n):
    return [(s, min(P, n - s)) for s in range(0, n, P)]


def build_program(PPs, Lp, Pb, live):
    import concourse.mybir as mybir
    import concourse.tile as tile
    from concourse import bacc

    f32 = mybir.dt.float32
    mdt = mybir.dt.float16
    PP0 = PPs[0]
    OUTW = max(PPs[N_ITERS], 1)

    nc = bacc.Bacc("TRN2", target_bir_lowering=False, debug=False,
                   num_devices=N_CORES)

    Lch = -(-Lp // P)
    Pbch = -(-Pb // P)

    # ---- DRAM parameters (all pre-chunked to [P, nch*C] by pack_rows) ----
    wnames = ["wlin", "wlc", "wlb", "wrsum", "wllp", "wrlp"]
    wd = {n: nc.declare_dram_parameter(n, [P, KC * D], mdt, isOutput=False)
          for n in wnames}
    bnames = ["blin", "blcb", "bllp"]
    bd = {n: nc.declare_dram_parameter(n, [1, D], mdt, isOutput=False)
          for n in bnames}
    ones_d = nc.declare_dram_parameter("ones", [1, 512], mdt, isOutput=False)
    vfT_d = nc.declare_dram_parameter("vfT", [P, KC * Lp], mdt, isOutput=False)
    subx_d = nc.declare_dram_parameter("subx", [P, Pbch * D], mdt, isOutput=False)
    subxT_d = nc.declare_dram_parameter("subxT", [P, KC * PP0], mdt,
                                        isOutput=False)
    mcompT_d = nc.declare_dram_parameter("mcompT", [P, Lch * PP0], mdt,
                                         isOutput=False)
    mcombT_d = nc.declare_dram_parameter("mcombT", [P, Pbch * PP0], mdt,
                                         isOutput=False)
    gt_d = {}
    for i in live:
        nch = -(-PPs[i] // P)
        gt_d[i] = nc.declare_dram_parameter(f"gt{i}", [P, nch * PPs[i + 1]],
                                            mdt, isOutput=False)
    out_d = nc.declare_dram_parameter("out", [KC, P, OUTW], f32, isOutput=True)

    with tile.TileContext(nc) as tc:
        with (
            tc.tile_pool(name="const", bufs=1) as const,
            tc.tile_pool(name="state", bufs=2) as state,
            tc.tile_pool(name="ps", bufs=4, space="PSUM") as ps,
        ):
            # ---- load constants, in first-use order ----
            # one dma_start per (packed) tensor, spread over the three
            # DMA-capable queues: sync + scalar (HWDGE) and gpsimd (SWDGE).
            # critical path (varx needs wlin/blin/ones/vfT) goes on sync.
            wt = {n: const.tile([P, KC * D], mdt, tag=n, name=n)
                  for n in wnames}
            bt = {n: const.tile([1, D], mdt, tag=n, name=n) for n in bnames}
            ones = const.tile([1, 512], mdt, tag="ones")

            nc.sync.dma_start(out=ones[:], in_=ones_d[:])
            # HAM warm-up: ~3.5us of dummy PE work issued while the
            # constant DMAs stream, so real matmuls start at 2.4 GHz
            for _ in range(8):
                wpt = ps.tile([P, 64], f32, tag="small_ps", name="warm")
                nc.tensor.matmul(out=wpt[:, :], lhsT=ones[:1, :P],
                                 rhs=ones[:1, :64], start=True, stop=True)
            nc.sync.dma_start(out=wt["wlin"][:], in_=wd["wlin"][:])
            nc.sync.dma_start(out=bt["blin"][:], in_=bd["blin"][:])
            vf_t = const.tile([P, KC * Lp], mdt, tag="vfT")
            nc.sync.dma_start(out=vf_t[:], in_=vfT_d[:])

            mcompT_t = const.tile([P, Lch * PP0], mdt, tag="mcompT")
            nc.scalar.dma_start(out=mcompT_t[:], in_=mcompT_d[:])
            subx_t = const.tile([P, Pbch * D], mdt, tag="subx")
            nc.scalar.dma_start(out=subx_t[:], in_=subx_d[:])
            mcombT_t = const.tile([P, Pbch * PP0], mdt, tag="mcombT")
            nc.scalar.dma_start(out=mcombT_t[:], in_=mcombT_d[:])
            for n in ("wlc", "wlb", "wrsum"):
                nc.scalar.dma_start(out=wt[n][:], in_=wd[n][:])
            nc.scalar.dma_start(out=bt["blcb"][:], in_=bd["blcb"][:])
            subxT_t = const.tile([P, KC * PP0], mdt, tag="subxT")
            nc.scalar.dma_start(out=subxT_t[:], in_=subxT_d[:])

            nc.gpsimd.dma_start(out=wt["wllp"][:], in_=wd["wllp"][:])
            nc.gpsimd.dma_start(out=wt["wrlp"][:], in_=wd["wrlp"][:])
            nc.gpsimd.dma_start(out=bt["bllp"][:], in_=bd["bllp"][:])
            gt_t = {}
            for i in live:
                nch = -(-PPs[i] // P)
                t = const.tile([P, nch * PPs[i + 1]], mdt, tag=f"gt{i}")
                nc.gpsimd.dma_start(out=t[:], in_=gt_d[i][:])
                gt_t[i] = t

            # PSUM-read copies round-robin between the vector (DVE) and
            # scalar (Activation) engines so chunk copies run concurrently.
            _cp = [0]

            def pcopy(out, in_):
                _cp[0] += 1
                if _cp[0] % 2:
                    nc.vector.tensor_copy(out=out, in_=in_)
                else:
                    nc.scalar.copy(out, in_)

            # ---- var_x = vfT.T @ W_lin + b_lin ----
            varx_t = []
            for ci, (s, sz) in enumerate(_chunks(Lp)):
                pt = ps.tile([P, D], f32, tag="big_ps")
                for k in range(KC):
                    nc.tensor.matmul(out=pt[:sz, :],
                                     lhsT=vf_t[:, k * Lp + s:k * Lp + s + sz],
                                     rhs=wt["wlin"][:, k * D:(k + 1) * D],
                                     start=(k == 0), stop=False)
                nc.tensor.matmul(out=pt[:sz, :], lhsT=ones[:1, :sz],
                                 rhs=bt["blin"][:1, :], start=False, stop=True)
                t = state.tile([P, D], mdt, tag=f"varx{ci}")
                pcopy(t[:sz, :], pt[:sz, :])
                varx_t.append(t)

            # ---- s_compT / s_combT (feature-major) ----
            scompT_t, scombT_t = [], []
            lpch = _chunks(Lp)
            pbch = _chunks(Pb)
            for dk in range(KC):
                pt = ps.tile([P, PP0], f32, tag="small_ps")
                for ci, (s, sz) in enumerate(lpch):
                    nc.tensor.matmul(out=pt[:, :],
                                     lhsT=varx_t[ci][:sz, dk * P:(dk + 1) * P],
                                     rhs=mcompT_t[:sz, ci * PP0:(ci + 1) * PP0],
                                     start=(ci == 0), stop=(ci == len(lpch) - 1))
                t = state.tile([P, PP0], mdt, tag=f"scompT{dk}")
                pcopy(t[:], pt[:])
                scompT_t.append(t)
            for dk in range(KC):
                pt = ps.tile([P, PP0], f32, tag="small_ps")
                for ci, (s, sz) in enumerate(pbch):
                    nc.tensor.matmul(out=pt[:, :],
                                     lhsT=subx_t[:sz, ci * D + dk * P:ci * D + (dk + 1) * P],
                                     rhs=mcombT_t[:sz, ci * PP0:(ci + 1) * PP0],
                                     start=(ci == 0), stop=(ci == len(pbch) - 1))
                t = state.tile([P, PP0], mdt, tag=f"scombT{dk}")
                pcopy(t[:], pt[:])
                scombT_t.append(t)

            # ---- x0T (one [P, KC*PP0] tile; dk chunks side by side) ----
            xTt = state.tile([P, KC * PP0], mdt, tag="xT_a")
            xw = PP0
            for dk in range(KC):
                pt = ps.tile([P, PP0], f32, tag="small_ps")
                for k in range(KC):
                    nc.tensor.matmul(out=pt[:, :],
                                     lhsT=wt["wlc"][:, k * D + dk * P:k * D + (dk + 1) * P],
                                     rhs=scompT_t[k][:, :],
                                     start=(k == 0), stop=False)
                for k in range(KC):
                    nc.tensor.matmul(out=pt[:, :],
                                     lhsT=wt["wlb"][:, k * D + dk * P:k * D + (dk + 1) * P],
                                     rhs=scombT_t[k][:, :],
                                     start=False, stop=False)
                for k in range(KC):
                    nc.tensor.matmul(out=pt[:, :],
                                     lhsT=wt["wrsum"][:, k * D + dk * P:k * D + (dk + 1) * P],
                                     rhs=subxT_t[:, k * PP0:(k + 1) * PP0],
                                     start=False, stop=False)
                nc.tensor.matmul(out=pt[:, :],
                                 lhsT=bt["blcb"][:1, dk * P:(dk + 1) * P],
                                 rhs=ones[:1, :PP0], start=False, stop=True)
                pcopy(xTt[:, dk * PP0:(dk + 1) * PP0], pt[:])

            # ---- iterations ----
            for step, i in enumerate(live):
                PPi, PPn = PPs[i], PPs[i + 1]
                uch = _chunks(PPi)

                # u = x @ Wl_lp  (natural layout, one psum+copy per chunk)
                us = [state.tile([P, D], mdt, tag=f"us{ci}",
                                 name=f"us{ci}_{step}")
                      for ci in range(len(uch))]
                for ci, (s, sz) in enumerate(uch):
                    pt = ps.tile([P, D], f32, tag="big_ps")
                    for k in range(KC):
                        nc.tensor.matmul(out=pt[:sz, :],
                                         lhsT=xTt[:, k * xw + s:k * xw + s + sz],
                                         rhs=wt["wllp"][:, k * D:(k + 1) * D],
                                         start=(k == 0), stop=(k == KC - 1))
                    pcopy(us[ci][:sz, :], pt[:sz, :])

                ab = "ab"[step % 2]
                # final step's tiles feed the f32 output DMA directly
                odt = f32 if step == len(live) - 1 else mdt
                # one psum bank holds all KC dk-chunks of the new xT.
                # self term (Wr_lp^T @ xT_prev) and bias accumulate directly
                # in xT layout -- no v/bias rows in us or G, and these
                # matmuls depend only on the previous state, so they run
                # while the u copies are still in flight.
                W = KC * PPn
                pt = ps.tile([P, W], f32, tag="small_ps")
                for dk in range(KC):
                    o = pt[:, dk * PPn:(dk + 1) * PPn]
                    for k in range(KC):
                        nc.tensor.matmul(
                            out=o,
                            lhsT=wt["wrlp"][:, k * D + dk * P:k * D + (dk + 1) * P],
                            rhs=xTt[:, k * xw:k * xw + PPn],
                            start=(k == 0), stop=False)
                    nc.tensor.matmul(out=o,
                                     lhsT=bt["bllp"][:1, dk * P:(dk + 1) * P],
                                     rhs=ones[:1, :PPn], start=False, stop=False)
                    for ci, (s, sz) in enumerate(uch):
                        nc.tensor.matmul(
                            out=o,
                            lhsT=us[ci][:sz, dk * P:(dk + 1) * P],
                            rhs=gt_t[i][:sz, ci * PPn:(ci + 1) * PPn],
                            start=False, stop=(ci == len(uch) - 1))
                xTt = state.tile([P, W], odt, tag=f"xT_{ab}")
                xw = PPn
                half = 2 * PPn  # split on a dk boundary: 2/3 vector, 1/3 scalar
                nc.vector.tensor_copy(out=xTt[:, :half], in_=pt[:, :half])
                nc.scalar.copy(xTt[:, half:], pt[:, half:])

            # ---- output ----
            for dk in range(KC):
                nc.sync.dma_start(out=out_d[dk],
                                  in_=xTt[:, dk * xw:dk * xw + OUTW])

    nc.compile()
    return nc


# ======================================================================
# Entry point
# ======================================================================

def kernel(**inputs) -> np.ndarray:
    out, _ = _run(inputs)
    return out


def _run(inputs, **spmd_kwargs):
    from concourse.bass_utils import run_bass_kernel_spmd

    pre = preprocess(inputs)
    PPs, Lp, Pb, live = pre["PPs"], pre["Lp"], pre["Pb"], pre["live"]

    nc = build_program(PPs, Lp, Pb, live)

    def f(a):
        return np.ascontiguousarray(np.asarray(a, F32))

    def h(a):
        return np.ascontiguousarray(np.asarray(a, F16))

    shared = {
        "wlin": h(inputs["W_lin"]), "wlc": h(inputs["Wl_c"]),
        "wlb": h(inputs["Wl_b"]),
        "wrsum": h(f(inputs["Wr_c"]) + f(inputs["Wr_b"])),
        "wllp": h(inputs["Wl_lp"]), "wrlp": h(inputs["Wr_lp"]),
        "blin": h(inputs["b_lin"]).reshape(1, D),
        "blcb": h(f(inputs["bl_c"]) + f(inputs["bl_b"])).reshape(1, D),
        "bllp": h(inputs["bl_lp"]).reshape(1, D),
        "ones": np.ones((1, 512), dtype=F16),
    }

    for k in ("wlin", "wlc", "wlb", "wrsum", "wllp", "wrlp"):
        shared[k] = pack_rows(shared[k])

    in_maps = []
    for c in range(N_CORES):
        prob = pre["probs"][c]
        m = dict(shared)
        m["vfT"] = pack_rows(h(prob["vfT"]))
        m["subx"] = pack_rows(h(prob["subx"]))
        m["subxT"] = pack_rows(h(prob["subxT"]))
        m["mcompT"] = pack_rows(h(prob["mcompT"]))
        m["mcombT"] = pack_rows(h(prob["mcombT"]))
        for i in live:
            m[f"gt{i}"] = pack_rows(h(prob["GTs"][i]))
        in_maps.append(m)

    res = run_bass_kernel_spmd(nc, in_maps, core_ids=list(range(N_CORES)),
                               **spmd_kwargs)

    B = len(pre["out_map"])
    OUTW = max(PPs[N_ITERS], 1)
    out = np.zeros((B, D), dtype=F32)
    for r, (c, row) in enumerate(pre["out_map"]):
        o = res.results[c]["out"].reshape(KC * P, OUTW)  # [384, OUTW]
        out[r] = o[:, row]
    return out, res



# revision 23
# speedup vs baseline: 1.4193x; 1.2817x over previous
"""Trainium2 Bass kernel for nn_AST_GAT (gnn_message_passing).

Strategy
--------
The module's output is only ``out[index_map[root_ids]]`` — 64 rows of the
65536-row node state after 20 mean-aggregation SAGE iterations over the
combine-edge forest.  The dependency closure of those 64 rows through the 20
iterations is computed on the host (pure index manipulation: leaf maps, edge
pruning dynamics, reverse BFS).  All sparsity (segment-sums, per-iteration
pruning masks, mean normalization) is folded into small dense selection /
normalized-adjacency matrices, so the device does only dense matmuls.

Sharding: the 64 roots are split 8-per-core (graph-parallel); each core's
closure is independent, so there is no cross-core traffic.  The host gathers
the 8x[8,384] results into the full [64,384] output.

Device program per core (state kept feature-major so no transposes are ever
needed; the self-term and bias are folded into an augmented adjacency matrix
G' = [G; I; 1] applied to the stacked [u; v; bias] block):
  var_x   = vfT.T @ W_lin + b_lin x 1
  s_compT = var_x.T @ McompT         s_combT = subx.T @ McombT
  x0T     = Wl_c.T @ s_compT + Wl_b.T @ s_combT
            + (Wr_c+Wr_b).T @ subxT + (bl_c+bl_b) x 1
  iter i:  u = x @ Wl_lp ; v = x[:PP'] @ Wr_lp   (natural layout, N=384 muls)
           xT' = [u; v; bl_lp].T @ G2T_i          (G2T = [GT; I; 1])
  output: xT20[:, :n_roots] -> host transposes.
"""
import sys

sys.path.insert(0, "/opt/trn_rl_repo")

import numpy as np

N_ITERS = 20
N_CORES = 8
D = 384
KC = 3  # 128-chunks of D
P = 128

F32 = np.float32
F16 = np.float16  # matmul operand dtype: fp16 streams 1 row/cycle on the PE
                  # (vs 4 for fp32) and halves DMA bytes; PSUM accumulates fp32


# ======================================================================
# Host-side preprocessing (faithful numpy reimplementation of the
# reference's index semantics + dependency closure of the root rows).
# ======================================================================

def _append_unique(order, pos, items):
    for s in items:
        s = int(s)
        if s not in pos:
            pos[s] = len(order)
            order.append(s)


def graph_prep(E, root_ids, n_var, n_sub):
    src = np.asarray(E[0], dtype=np.int64)
    dst = np.asarray(E[1], dtype=np.int64)
    root_ids = np.asarray(root_ids, dtype=np.int64)
    N = n_var + n_sub

    leaf_mask = np.ones(N, dtype=bool)
    leaf_mask[dst] = False
    leaf_idx = np.cumsum(leaf_mask) - 1
    nleaf_idx = np.cumsum(~leaf_mask) - 1
    index_map = np.where(leaf_mask, leaf_idx, nleaf_idx)

    src_is_leaf = leaf_mask[src]
    src_loc = index_map[src]
    dst_loc = index_map[dst]
    src_s = np.clip(src_loc, 0, n_sub - 1)

    # simulate the pruning dynamics exactly as the reference scan does
    actives, dones = [], []
    active = ~src_is_leaf
    done = False
    for _ in range(N_ITERS):
        actives.append(active.copy())
        dones.append(done)
        idx = np.where(active, dst_loc, n_sub)
        valid = (idx >= 0) & (idx <= n_sub)  # jax scatter drops OOB
        is_tgt = np.zeros(n_sub + 1, dtype=bool)
        is_tgt[idx[valid]] = True
        active_new = active & is_tgt[src_s]
        if not done:
            active = active_new
        done = done or (not active.any())

    dmask = (dst_loc >= 0) & (dst_loc < n_sub)
    cnts = []
    for i in range(N_ITERS):
        if dones[i]:
            cnts.append(None)
        else:
            d = dst_loc[actives[i] & dmask]
            cnts.append(np.bincount(d, minlength=n_sub).astype(np.float64))

    return dict(
        src_is_leaf=src_is_leaf, src_loc=src_loc, dst_loc=dst_loc, src_s=src_s,
        actives=actives, dones=dones, cnts=cnts, dmask=dmask,
        j_roots=index_map[root_ids], n_sub=n_sub,
    )


def core_closure(g, roots):
    dst_loc, src_s = g["dst_loc"], g["src_s"]
    actives, dones, n_sub, dmask = g["actives"], g["dones"], g["n_sub"], g["dmask"]

    order, pos = [], {}
    _append_unique(order, pos, roots)
    lens = [0] * (N_ITERS + 1)
    lens[N_ITERS] = len(order)

    member = np.zeros(n_sub, dtype=bool)
    member[order] = True

    for i in range(N_ITERS - 1, -1, -1):
        if not dones[i]:
            e = actives[i] & dmask
            e[e] = member[dst_loc[e]]
            fresh = np.unique(src_s[e])
            fresh = fresh[~member[fresh]]
            _append_unique(order, pos, np.sort(fresh))
            member[fresh] = True
        lens[i] = len(order)

    posarr = np.full(n_sub, -1, dtype=np.int64)
    order_arr = np.array(order, dtype=np.int64)
    posarr[order_arr] = np.arange(len(order))

    in0 = dmask.copy()
    in0[dmask] = member[dst_loc[dmask]]

    comp_e = in0 & g["src_is_leaf"]
    leaves = g["src_loc"][comp_e]
    leaf_order, leaf_pos = [], {}
    _append_unique(leaf_order, leaf_pos, leaves)

    comb_e = in0 & ~g["src_is_leaf"]
    subs = np.clip(g["src_loc"][comb_e], 0, n_sub - 1)  # jax gather clamps
    sub_order, sub_pos = list(order), dict(pos)
    _append_unique(sub_order, sub_pos, subs)

    return dict(
        order=order_arr, pos=pos, posarr=posarr, lens=lens,
        comp_e=comp_e, leaves=leaves, leaf_order=np.array(leaf_order, np.int64),
        leaf_pos=leaf_pos,
        comb_e=comb_e, subs=subs, sub_order=np.array(sub_order, np.int64),
        sub_pos=sub_pos,
    )


def build_core_problem(g, cl, inputs, PPs, Lp, Pb):
    dst_loc, src_s = g["dst_loc"], g["src_s"]
    actives, dones, cnts, dmask = g["actives"], g["dones"], g["cnts"], g["dmask"]
    posarr, lens = cl["posarr"], cl["lens"]
    n0, PP0 = lens[0], PPs[0]
    n_sub = g["n_sub"]

    McompT = np.zeros((Lp, PP0), dtype=F32)
    if cl["leaves"].size:
        lcols = np.array([cl["leaf_pos"][int(s)] for s in cl["leaves"]], np.int64)
        np.add.at(McompT, (lcols, posarr[dst_loc[cl["comp_e"]]]), 1.0)

    McombT = np.zeros((Pb, PP0), dtype=F32)
    if cl["subs"].size:
        scols = np.array([cl["sub_pos"][int(s)] for s in cl["subs"]], np.int64)
        np.add.at(McombT, (scols, posarr[dst_loc[cl["comb_e"]]]), 1.0)

    GTs = []
    for i in range(N_ITERS):
        if dones[i]:
            GTs.append(None)
            continue
        nip1 = lens[i + 1]
        e = actives[i] & dmask
        sel = e.copy()
        p = posarr[dst_loc[e]]
        sel[e] = (p >= 0) & (p < nip1)
        GT = np.zeros((PPs[i], PPs[i + 1]), dtype=F32)
        if sel.any():
            rows = posarr[dst_loc[sel]]
            cols = posarr[src_s[sel]]
            vals = (1.0 / np.maximum(cnts[i][dst_loc[sel]], 1.0)).astype(F32)
            np.add.at(GT, (cols, rows), vals)
        GTs.append(GT)

    var_feats = np.asarray(inputs["var_feats"], dtype=F32)
    code_emb = np.asarray(inputs["code_emb"], dtype=F32)
    sids = np.clip(np.asarray(inputs["subcode_ids"], dtype=np.int64), 0,
                   code_emb.shape[0] - 1)

    vfT = np.zeros((D, Lp), dtype=F32)
    lo = cl["leaf_order"]
    if lo.size:
        vfT[:, : lo.size] = var_feats[lo].T
    subx = np.zeros((Pb, D), dtype=F32)
    so = cl["sub_order"]
    subx[: so.size] = code_emb[sids[so]]
    subxT_pref = np.zeros((D, PP0), dtype=F32)
    subxT_pref[:, :n0] = subx[:n0].T

    return dict(vfT=vfT, subx=subx, subxT=subxT_pref,
                mcompT=McompT, mcombT=McombT, GTs=GTs)


def preprocess(inputs):
    n_var = inputs["var_feats"].shape[0]
    n_sub = inputs["subcode_ids"].shape[0]
    root_ids = np.asarray(inputs["root_ids"], dtype=np.int64)
    B = root_ids.shape[0]
    assert B % N_CORES == 0
    per_core = B // N_CORES

    g = graph_prep(np.asarray(inputs["E"]), root_ids, n_var, n_sub)
    closures = [core_closure(g, g["j_roots"][c * per_core:(c + 1) * per_core])
                for c in range(N_CORES)]

    # no padding needed: every copy/matmul now starts at partition 0
    PPs = [max(cl["lens"][i] for cl in closures) for i in range(N_ITERS + 1)]
    Lp = max(max(cl["leaf_order"].size for cl in closures), 1)
    Pb = max(max(cl["sub_order"].size for cl in closures), 1)

    probs = [build_core_problem(g, cl, inputs, PPs, Lp, Pb) for cl in closures]

    out_map = []
    for r in range(B):
        c = r // per_core
        j = int(g["j_roots"][r])
        out_map.append((c, closures[c]["pos"][j]))

    live = [i for i in range(N_ITERS) if not g["dones"][i]]
    return dict(probs=probs, PPs=PPs, Lp=Lp, Pb=Pb, out_map=out_map, live=live)


def pack_rows(a):
    """[R, C] -> [P, ceil(R/P)*C]: 128-row chunks laid side by side, so a
    whole tensor lands in SBUF with ONE dma_start (DMA issue on the engines
    is serial and ~600ns per descriptor — fewer, bigger transfers win)."""
    R, C = a.shape
    nch = -(-R // P)
    out = np.zeros((P, nch * C), a.dtype)
    for c in range(nch):
        r0, r1 = c * P, min((c + 1) * P, R)
        out[: r1 - r0, c * C:(c + 1) * C] = a[r0:r1]
    return out


# ======================================================================
# Device program
# ======================================================================

def _chunks(n):
    return [(s, min(P, n - s)) for s in range(0, n, P)]


def build_program(PPs, Lp, Pb, live):
    import concourse.mybir as mybir
    import concourse.tile as tile
    from concourse import bacc

    f32 = mybir.dt.float32
    mdt = mybir.dt.float16
    PP0 = PPs[0]
    OUTW = max(PPs[N_ITERS], 1)

    nc = bacc.Bacc("TRN2", target_bir_lowering=False, debug=False,
                   num_devices=N_CORES)

    Lch = -(-Lp // P)
    Pbch = -(-Pb // P)

    # ---- DRAM parameters (all pre-chunked to [P, nch*C] by pack_rows) ----
    wnames = ["wlin", "wlc", "wlb", "wrsum", "wllp", "wrlp"]
    wd = {n: nc.declare_dram_parameter(n, [P, KC * D], mdt, isOutput=False)
          for n in wnames}
    bnames = ["blin", "blcb", "bllp"]
    bd = {n: nc.declare_dram_parameter(n, [1, D], mdt, isOutput=False)
          for n in bnames}
    ones_d = nc.declare_dram_parameter("ones", [1, 512], mdt, isOutput=False)
    vfT_d = nc.declare_dram_parameter("vfT", [P, KC * Lp], mdt, isOutput=False)
    subx_d = nc.declare_dram_parameter("subx", [P, Pbch * D], mdt, isOutput=False)
    subxT_d = nc.declare_dram_parameter("subxT", [P, KC * PP0], mdt,
                                        isOutput=False)
    mcompT_d = nc.declare_dram_parameter("mcompT", [P, Lch * PP0], mdt,
                                         isOutput=False)
    mcombT_d = nc.declare_dram_parameter("mcombT", [P, Pbch * PP0], mdt,
                                         isOutput=False)
    gt_d = {}
    for i in live:
        nch = -(-PPs[i] // P)
        gt_d[i] = nc.declare_dram_parameter(f"gt{i}", [P, nch * PPs[i + 1]],
                                            mdt, isOutput=False)
    out_d = nc.declare_dram_parameter("out", [KC, P, OUTW], f32, isOutput=True)

    with tile.TileContext(nc) as tc:
        with (
            tc.tile_pool(name="const", bufs=1) as const,
            tc.tile_pool(name="state", bufs=2) as state,
            tc.tile_pool(name="ps", bufs=2, space="PSUM") as ps,
            tc.tile_pool(name="ps2", bufs=1, space="PSUM") as ps2,
        ):
            # ---- load constants, in first-use order ----
            # one dma_start per (packed) tensor, spread over the three
            # DMA-capable queues: sync + scalar (HWDGE) and gpsimd (SWDGE).
            # critical path (varx needs wlin/blin/ones/vfT) goes on sync.
            wt = {n: const.tile([P, KC * D], mdt, tag=n, name=n)
                  for n in wnames}
            bt = {n: const.tile([1, D], mdt, tag=n, name=n) for n in bnames}
            ones = const.tile([1, 512], mdt, tag="ones")

            nc.sync.dma_start(out=ones[:], in_=ones_d[:])
            # HAM warm-up: ~3.5us of dummy PE work issued while the
            # constant DMAs stream, so real matmuls start at 2.4 GHz
            for _ in range(8):
                wpt = ps.tile([P, 64], f32, tag="small_ps", name="warm")
                nc.tensor.matmul(out=wpt[:, :], lhsT=ones[:1, :P],
                                 rhs=ones[:1, :64], start=True, stop=True)
            nc.sync.dma_start(out=wt["wlin"][:], in_=wd["wlin"][:])
            nc.sync.dma_start(out=bt["blin"][:], in_=bd["blin"][:])
            vf_t = const.tile([P, KC * Lp], mdt, tag="vfT")
            nc.sync.dma_start(out=vf_t[:], in_=vfT_d[:])

            mcompT_t = const.tile([P, Lch * PP0], mdt, tag="mcompT")
            nc.scalar.dma_start(out=mcompT_t[:], in_=mcompT_d[:])
            subx_t = const.tile([P, Pbch * D], mdt, tag="subx")
            nc.scalar.dma_start(out=subx_t[:], in_=subx_d[:])
            mcombT_t = const.tile([P, Pbch * PP0], mdt, tag="mcombT")
            nc.scalar.dma_start(out=mcombT_t[:], in_=mcombT_d[:])
            for n in ("wlc", "wlb", "wrsum"):
                nc.scalar.dma_start(out=wt[n][:], in_=wd[n][:])
            nc.scalar.dma_start(out=bt["blcb"][:], in_=bd["blcb"][:])
            subxT_t = const.tile([P, KC * PP0], mdt, tag="subxT")
            nc.scalar.dma_start(out=subxT_t[:], in_=subxT_d[:])

            nc.gpsimd.dma_start(out=wt["wllp"][:], in_=wd["wllp"][:])
            nc.gpsimd.dma_start(out=wt["wrlp"][:], in_=wd["wrlp"][:])
            nc.gpsimd.dma_start(out=bt["bllp"][:], in_=bd["bllp"][:])
            gt_t = {}
            for i in live:
                nch = -(-PPs[i] // P)
                t = const.tile([P, nch * PPs[i + 1]], mdt, tag=f"gt{i}")
                nc.gpsimd.dma_start(out=t[:], in_=gt_d[i][:])
                gt_t[i] = t

            # PSUM-read copies round-robin between the vector (DVE) and
            # scalar (Activation) engines so chunk copies run concurrently.
            _cp = [0]

            def pcopy(out, in_):
                _cp[0] += 1
                if _cp[0] % 2:
                    nc.vector.tensor_copy(out=out, in_=in_)
                else:
                    nc.scalar.copy(out, in_)

            # ---- var_x = vfT.T @ W_lin + b_lin ----
            varx_t = []
            for ci, (s, sz) in enumerate(_chunks(Lp)):
                pt = ps.tile([P, D], f32, tag="big_ps")
                for k in range(KC):
                    nc.tensor.matmul(out=pt[:sz, :],
                                     lhsT=vf_t[:, k * Lp + s:k * Lp + s + sz],
                                     rhs=wt["wlin"][:, k * D:(k + 1) * D],
                                     start=(k == 0), stop=False)
                nc.tensor.matmul(out=pt[:sz, :], lhsT=ones[:1, :sz],
                                 rhs=bt["blin"][:1, :], start=False, stop=True)
                t = state.tile([P, D], mdt, tag=f"varx{ci}")
                pcopy(t[:sz, :], pt[:sz, :])
                varx_t.append(t)

            # ---- s_compT / s_combT (feature-major) ----
            scompT_t, scombT_t = [], []
            lpch = _chunks(Lp)
            pbch = _chunks(Pb)
            for dk in range(KC):
                pt = ps.tile([P, PP0], f32, tag="small_ps")
                for ci, (s, sz) in enumerate(lpch):
                    nc.tensor.matmul(out=pt[:, :],
                                     lhsT=varx_t[ci][:sz, dk * P:(dk + 1) * P],
                                     rhs=mcompT_t[:sz, ci * PP0:(ci + 1) * PP0],
                                     start=(ci == 0), stop=(ci == len(lpch) - 1))
                t = state.tile([P, PP0], mdt, tag=f"scompT{dk}")
                pcopy(t[:], pt[:])
                scompT_t.append(t)
            for dk in range(KC):
                pt = ps.tile([P, PP0], f32, tag="small_ps")
                for ci, (s, sz) in enumerate(pbch):
                    nc.tensor.matmul(out=pt[:, :],
                                     lhsT=subx_t[:sz, ci * D + dk * P:ci * D + (dk + 1) * P],
                                     rhs=mcombT_t[:sz, ci * PP0:(ci + 1) * PP0],
                                     start=(ci == 0), stop=(ci == len(pbch) - 1))
                t = state.tile([P, PP0], mdt, tag=f"scombT{dk}")
                pcopy(t[:], pt[:])
                scombT_t.append(t)

            # ---- x0T (one [P, KC*PP0] tile; dk chunks side by side) ----
            xTt = state.tile([P, KC * PP0], mdt, tag="xT_a")
            xw = PP0
            for dk in range(KC):
                pt = ps.tile([P, PP0], f32, tag="small_ps")
                for k in range(KC):
                    nc.tensor.matmul(out=pt[:, :],
                                     lhsT=wt["wlc"][:, k * D + dk * P:k * D + (dk + 1) * P],
                                     rhs=scompT_t[k][:, :],
                                     start=(k == 0), stop=False)
                for k in range(KC):
                    nc.tensor.matmul(out=pt[:, :],
                                     lhsT=wt["wlb"][:, k * D + dk * P:k * D + (dk + 1) * P],
                                     rhs=scombT_t[k][:, :],
                                     start=False, stop=False)
                for k in range(KC):
                    nc.tensor.matmul(out=pt[:, :],
                                     lhsT=wt["wrsum"][:, k * D + dk * P:k * D + (dk + 1) * P],
                                     rhs=subxT_t[:, k * PP0:(k + 1) * PP0],
                                     start=False, stop=False)
                nc.tensor.matmul(out=pt[:, :],
                                 lhsT=bt["blcb"][:1, dk * P:(dk + 1) * P],
                                 rhs=ones[:1, :PP0], start=False, stop=True)
                pcopy(xTt[:, dk * PP0:(dk + 1) * PP0], pt[:])

            # ---- iterations ----
            for step, i in enumerate(live):
                PPi, PPn = PPs[i], PPs[i + 1]
                uch = _chunks(PPi)

                # u = x @ Wl_lp  (natural layout, one psum+copy per chunk)
                us = [state.tile([P, D], mdt, tag=f"us{ci}",
                                 name=f"us{ci}_{step}")
                      for ci in range(len(uch))]
                for ci, (s, sz) in enumerate(uch):
                    pt = ps.tile([P, D], f32, tag="big_ps")
                    for k in range(KC):
                        nc.tensor.matmul(out=pt[:sz, :],
                                         lhsT=xTt[:, k * xw + s:k * xw + s + sz],
                                         rhs=wt["wllp"][:, k * D:(k + 1) * D],
                                         start=(k == 0), stop=(k == KC - 1))
                    pcopy(us[ci][:sz, :], pt[:sz, :])

                ab = "ab"[step % 2]
                # final step's tiles feed the f32 output DMA directly
                odt = f32 if step == len(live) - 1 else mdt
                # per-dk psum banks (separate accumulation groups, so the
                # self-term/bias matmuls for ALL dk can be issued before the
                # GT matmuls -- they depend only on the previous state and
                # keep the PE busy while the u copies are in flight).
                pts = [ps2.tile([P, PPn], f32, tag=f"xps{dk}",
                                name=f"xps{dk}_{step}") for dk in range(KC)]
                for dk in range(KC):
                    for k in range(KC):
                        nc.tensor.matmul(
                            out=pts[dk][:, :],
                            lhsT=wt["wrlp"][:, k * D + dk * P:k * D + (dk + 1) * P],
                            rhs=xTt[:, k * xw:k * xw + PPn],
                            start=(k == 0), stop=False)
                    nc.tensor.matmul(out=pts[dk][:, :],
                                     lhsT=bt["bllp"][:1, dk * P:(dk + 1) * P],
                                     rhs=ones[:1, :PPn], start=False, stop=False)
                xTt = state.tile([P, KC * PPn], odt, tag=f"xT_{ab}")
                xw = PPn
                for dk in range(KC):
                    for ci, (s, sz) in enumerate(uch):
                        nc.tensor.matmul(
                            out=pts[dk][:, :],
                            lhsT=us[ci][:sz, dk * P:(dk + 1) * P],
                            rhs=gt_t[i][:sz, ci * PPn:(ci + 1) * PPn],
                            start=False, stop=(ci == len(uch) - 1))
                    # copy each dk slice as soon as its group closes, so the
                    # next iteration's k=dk matmuls can start early
                    if dk == 1:
                        nc.scalar.copy(xTt[:, dk * PPn:(dk + 1) * PPn],
                                       pts[dk][:, :])
                    else:
                        nc.vector.tensor_copy(
                            out=xTt[:, dk * PPn:(dk + 1) * PPn],
                            in_=pts[dk][:, :])

            # ---- output ----
            for dk in range(KC):
                nc.sync.dma_start(out=out_d[dk],
                                  in_=xTt[:, dk * xw:dk * xw + OUTW])

    nc.compile()
    return nc


# ======================================================================
# Entry point
# ======================================================================

def kernel(**inputs) -> np.ndarray:
    out, _ = _run(inputs)
    return out


def _run(inputs, **spmd_kwargs):
    from concourse.bass_utils import run_bass_kernel_spmd

    pre = preprocess(inputs)
    PPs, Lp, Pb, live = pre["PPs"], pre["Lp"], pre["Pb"], pre["live"]

    nc = build_program(PPs, Lp, Pb, live)

    def f(a):
        return np.ascontiguousarray(np.asarray(a, F32))

    def h(a):
        return np.ascontiguousarray(np.asarray(a, F16))

    shared = {
        "wlin": h(inputs["W_lin"]), "wlc": h(inputs["Wl_c"]),
        "wlb": h(inputs["Wl_b"]),
        "wrsum": h(f(inputs["Wr_c"]) + f(inputs["Wr_b"])),
        "wllp": h(inputs["Wl_lp"]), "wrlp": h(inputs["Wr_lp"]),
        "blin": h(inputs["b_lin"]).reshape(1, D),
        "blcb": h(f(inputs["bl_c"]) + f(inputs["bl_b"])).reshape(1, D),
        "bllp": h(inputs["bl_lp"]).reshape(1, D),
        "ones": np.ones((1, 512), dtype=F16),
    }

    for k in ("wlin", "wlc", "wlb", "wrsum", "wllp", "wrlp"):
        shared[k] = pack_rows(shared[k])

    in_maps = []
    for c in range(N_CORES):
        prob = pre["probs"][c]
        m = dict(shared)
        m["vfT"] = pack_rows(h(prob["vfT"]))
        m["subx"] = pack_rows(h(prob["subx"]))
        m["subxT"] = pack_rows(h(prob["subxT"]))
        m["mcompT"] = pack_rows(h(prob["mcompT"]))
        m["mcombT"] = pack_rows(h(prob["mcombT"]))
        for i in live:
            m[f"gt{i}"] = pack_rows(h(prob["GTs"][i]))
        in_maps.append(m)

    res = run_bass_kernel_spmd(nc, in_maps, core_ids=list(range(N_CORES)),
                               **spmd_kwargs)

    B = len(pre["out_map"])
    OUTW = max(PPs[N_ITERS], 1)
    out = np.zeros((B, D), dtype=F32)
    for r, (c, row) in enumerate(pre["out_map"]):
        o = res.results[c]["out"].reshape(KC * P, OUTW)  # [384, OUTW]
        out[r] = o[:, row]
    return out, res

